# revision 6
# baseline (speedup 1.0000x reference)
"""Trainium2 Bass kernel for nn_BertL2PredictionHead: out = -||x - emb||_2 + bias.

out[b,s,v] = bias[v] - sqrt(max(||x_bs||^2 + ||emb_v||^2 - 2 x_bs.emb_v, 0))
for x (16,128,128) f32, emb (20001,128) f32, bias (1,1,20001) f32.

Sharding: vocab dimension split across 8 NeuronCores (tensor parallel over V),
x replicated. Per core, for each 128-row tile of the 2048x2502 slice:
  psum = (-2 x^T)^T @ embT          f32r (TF32) matmul, 1 cyc/col, same
                                    stationary weights for all 5 chunks
  o    = psum + esq_bcast           DVE tensor_tensor, adds ||emb_v||^2 (fp32)
  o    = Sqrt(o + ||x_m||^2)        one wide ACT per m-tile, per-partition bias
  DMA o -> out slice                one contiguous 1.28 MB store per m-tile
The final negation and the bias add are fused into the host-side gather copy
(np.negative / np.subtract with out=), which costs nothing beyond the copy.
"""
import sys

sys.path.insert(0, "/opt/trn_rl_repo")

import numpy as np
from contextlib import ExitStack

import concourse.bass as bass  # noqa: F401
import concourse.tile as tile
from concourse import bacc, mybir
from concourse.bass_utils import run_bass_kernel_spmd

F32 = mybir.dt.float32
F32R = mybir.dt.float32r

NCORES = 8
B, S, H, V = 16, 128, 128, 20001
BS = B * S                      # 2048 rows
MT = BS // 128                  # 16 m-tiles of 128 rows
VS = 2502                       # vocab slice per core (even: fp32r needs even free dims)
VPAD = VS * NCORES              # 20016
# psum tiles per m-tile: two [128,1024] (2 banks each) + one [128,454]
PW = 1024
TAIL = VS - 2 * PW              # 454


def _tf32(a: np.ndarray) -> np.ndarray:
    """Round fp32 to TF32 (10-bit mantissa, round-to-nearest-even)."""
    u = a.view(np.uint32).astype(np.uint64)
    lsb = (u >> 13) & 1
    u2 = (u + 0x0FFF + lsb) & 0xFFFFFFFF
    return (u2 & ~np.uint64(0x1FFF)).astype(np.uint32).view(np.float32)


_PROG = None  # (nc,) compiled once per process


def _build():
    global _PROG
    if _PROG is not None:
        return _PROG

    nc = bacc.Bacc("TRN2", target_bir_lowering=False, debug=False)

    xT2_d = nc.dram_tensor("xT2", [H, BS], F32R, kind="ExternalInput").ap()
    embT_d = nc.dram_tensor("embT", [H, VS], F32R, kind="ExternalInput").ap()
    esq_d = nc.dram_tensor("esq", [1, VS], F32, kind="ExternalInput").ap()
    xsqc_d = nc.dram_tensor("xsqc", [128, MT], F32, kind="ExternalInput").ap()
    out_d = nc.dram_tensor("out", [BS, VS], F32, kind="ExternalOutput").ap()

    with tile.TileContext(nc) as tc, ExitStack() as ctx:
        const = ctx.enter_context(tc.tile_pool(name="const", bufs=1))
        opool = ctx.enter_context(tc.tile_pool(name="opool", bufs=4))
        psum = ctx.enter_context(tc.tile_pool(name="psum", bufs=1, space="PSUM"))

        # Staged inputs, split so the first matmuls gate on ~320KB only
        # (subtile deps track per-slice DMA completion).
        xt_s = const.tile([H, BS], F32R)
        emb_s = const.tile([H, VS], F32R)
        nc.sync.dma_start(out=xt_s[:, 0:128], in_=xT2_d[:, 0:128])
        nc.sync.dma_start(out=emb_s[:, 0:512], in_=embT_d[:, 0:512])
        nc.sync.dma_start(out=xt_s[:, 128:BS], in_=xT2_d[:, 128:BS])
        nc.sync.dma_start(out=emb_s[:, 512:VS], in_=embT_d[:, 512:VS])
        xsqc_s = const.tile([128, MT], F32)
        nc.sync.dma_start(out=xsqc_s[:], in_=xsqc_d[:])
        # esq broadcast to all 128 partitions (one-time, replicating DMA)
        esqb = const.tile([128, VS], F32)
        nc.sync.dma_start(out=esqb[:], in_=esq_d[:].broadcast_to([128, VS]))

        for t in range(MT):
            o_t = opool.tile([128, VS], F32, tag="o", name=f"o{t}")
            xt = xt_s[:, t * 128:(t + 1) * 128]
            for g in range(2):
                pw = psum.tile([128, PW], F32, tag="pw", bufs=3, name=f"pw{t}_{g}")
                for h in range(2):
                    c0 = g * PW + h * 512
                    nc.tensor.matmul(pw[:, h * 512:(h + 1) * 512], xt,
                                     emb_s[:, c0:c0 + 512], start=True, stop=True)
                nc.vector.tensor_add(o_t[:, g * PW:(g + 1) * PW], pw[:],
                                     esqb[:, g * PW:(g + 1) * PW])
            pt = psum.tile([128, TAIL], F32, tag="pt", bufs=2, name=f"pt{t}")
            nc.tensor.matmul(pt[:], xt, emb_s[:, 2 * PW:VS], start=True, stop=True)
            nc.vector.tensor_add(o_t[:, 2 * PW:VS], pt[:], esqb[:, 2 * PW:VS])

            rows = out_d[t * 128:(t + 1) * 128, :]
            if t < MT - 1:
                nc.scalar.activation(o_t[:], o_t[:],
                                     mybir.ActivationFunctionType.Sqrt,
                                     bias=xsqc_s[:, t:t + 1], scale=1.0)
                nc.sync.dma_start(out=rows, in_=o_t[:])
            else:
                # last tile: chunked so the final store is small (short tail)
                for (c0, c1) in ((0, PW), (PW, 2 * PW), (2 * PW, VS)):
                    nc.scalar.activation(o_t[:, c0:c1], o_t[:, c0:c1],
                                         mybir.ActivationFunctionType.Sqrt,
                                         bias=xsqc_s[:, t:t + 1], scale=1.0)
                    nc.sync.dma_start(out=rows[:, c0:c1], in_=o_t[:, c0:c1])

    nc.compile()
    _PROG = (nc,)
    return _PROG


def _prep_in_maps(x: np.ndarray, emb: np.ndarray):
    X = np.asarray(x, dtype=np.float32).reshape(BS, H)
    xT2 = _tf32(np.ascontiguousarray(X.T) * np.float32(-2.0))
    xsq = (X.astype(np.float64) ** 2).sum(axis=1).astype(np.float32)
    xsqc = np.ascontiguousarray(xsq.reshape(MT, 128).T)   # [128, MT]

    embp = np.zeros((VPAD, H), dtype=np.float32)
    embp[:V] = np.asarray(emb, dtype=np.float32)
    embT = _tf32(np.ascontiguousarray(embp.T))            # [H, VPAD]
    esq = (embp.astype(np.float64) ** 2).sum(axis=1).astype(np.float32)

    maps = []
    for c in range(NCORES):
        lo = c * VS
        maps.append({
            "xT2": xT2,
            "embT": np.ascontiguousarray(embT[:, lo:lo + VS]),
            "esq": np.ascontiguousarray(esq[lo:lo + VS].reshape(1, VS)),
            "xsqc": xsqc,
        })
    return maps


def _run_cores(in_maps, trace: bool = False):
    (nc,) = _build()
    return run_bass_kernel_spmd(nc, in_maps, list(range(NCORES)), trace=trace)


def kernel(x: np.ndarray, emb: np.ndarray, bias: np.ndarray) -> np.ndarray:
    in_maps = _prep_in_maps(x, emb)
    res = _run_cores(in_maps)

    bias_np = np.asarray(bias, dtype=np.float32).reshape(-1)
    have_bias = bool(np.any(bias_np))

    # Gather + fused negate (+ bias): out = bias - dist
    out = np.empty((BS, V), dtype=np.float32)
    for c in range(NCORES):
        lo = c * VS
        hi = min(lo + VS, V)
        dist = res.results[c]["out"][:, :hi - lo]
        if have_bias:
            np.subtract(bias_np[lo:hi][None, :], dist, out=out[:, lo:hi])
        else:
            np.negative(dist, out=out[:, lo:hi])
    return out.reshape(B, S, V)
